# revision 15
# baseline (speedup 1.0000x reference)
"""
Trainium2 Bass kernel for nn_AllInOneFlow (affine coupling flow).

Math folding (host side, exact):
  - inv_perm[perm[j]] == j  =>  the input gather and output scatter cancel:
    output column j reads input column j.  All permutation effects fold into
    small constant matrices applied via the PE:
      W1e  [65,256] : rows 0:64 = W1 rows scattered to inv_perm[:32] (rest 0),
                      row 64 = b1 (bias via ones-row trick)
      SWS  [33,64]  : Sg  = SWS.T @ [S_c; ones],  S_c = exp(2*tanh_t + gls[32+t])
      SWO  [33,64]  : OFFg= SWO.T @ [po; ones]
    y[:,j] = Sg[j]*x[:,j] + OFFg[j];  log_jac = sum_t 2*tanh_t + sum(gls)

Device layout: activations feature-major [feat, batch]; batch streamed in
tiles of 512 columns; 8 cores pure data-parallel over the batch.
Phase 1 (gelu/tanh ACT table set) computes the MLP + tanh + logjac for all
tiles; Phase 2 (exp table set) does exp, output matmuls, final elementwise,
and transposes back to batch-major.
"""

import os
import numpy as np

import concourse.bass as bass
import concourse.bacc as bacc_mod
import concourse.tile as tile
from concourse import mybir
from concourse.bass_utils import run_bass_kernel_spmd

# Problem constants (hardcoded per harness contract)
B, D, H = 131072, 64, 256
D1 = D // 2
D2 = D - D1
ALPHA = 2.0
NCORES = 8
BL = B // NCORES          # rows per core = 16384
NT = 512                  # batch columns per tile
TILES = BL // NT          # 32
NP = 128                  # n-dim of the (n p) partition split
NCH = NT // NP            # 4 transpose chunks per tile

F32 = mybir.dt.float32
F32R = mybir.dt.float32r

# Tunables: DT is the dtype of everything on the matmul path
# (F32 = exact 4cyc/row, F32R = 1cyc/row at N>=256)
DT = F32R


def _mm(ap):
    return ap


def _tr(ap):
    return ap


def build_program():
    nc = bacc_mod.Bacc("TRN2", target_bir_lowering=False, debug=False)

    x_d = nc.declare_dram_parameter("x", [BL, D], DT, isOutput=False)
    w1e_d = nc.declare_dram_parameter("W1e", [65, H], DT, isOutput=False)
    w2_d = nc.declare_dram_parameter("W2", [H, H], DT, isOutput=False)
    b2_d = nc.declare_dram_parameter("b2", [H], F32, isOutput=False)
    w3_d = nc.declare_dram_parameter("W3r", [H, D], DT, isOutput=False)
    b3t_d = nc.declare_dram_parameter("b3t", [D2], F32, isOutput=False)
    glsp_d = nc.declare_dram_parameter("glsp", [D2], F32, isOutput=False)
    sws_d = nc.declare_dram_parameter("SWS", [33, D], DT, isOutput=False)
    swo_d = nc.declare_dram_parameter("SWO", [33, D], DT, isOutput=False)
    ident_d = nc.declare_dram_parameter("ident", [128, 128], DT, isOutput=False)
    ones_d = nc.declare_dram_parameter("ones", [BL], DT, isOutput=False)
    lj2_d = nc.declare_dram_parameter("lj2", [D2], DT, isOutput=False)
    ljc_d = nc.declare_dram_parameter("ljc", [1], F32, isOutput=False)

    y_d = nc.declare_dram_parameter("y", [BL, D], F32, isOutput=True)
    lj_d = nc.declare_dram_parameter("lj", [1, BL], F32, isOutput=True)

    # batch index = n*128 + p  ->  partition p, chunk n
    x_ap = x_d[:].rearrange("(n p) d -> p n d", p=NP)
    y_ap = y_d[:].rearrange("(n p) d -> p n d", p=NP)

    gelu_f = mybir.ActivationFunctionType.Gelu_apprx_tanh
    tanh_f = mybir.ActivationFunctionType.Tanh
    exp_f = mybir.ActivationFunctionType.Exp

    with tile.TileContext(nc) as tc:
        with (
            tc.tile_pool(name="consts", bufs=1) as consts,
            tc.tile_pool(name="persist", bufs=1) as persist,
        ):
            ident = consts.tile([128, 128], DT)
            w1e = consts.tile([65, 2, 128], DT)       # [k, mchunk, m]
            w2 = consts.tile([128, 2, 2, 128], DT)    # [k, kchunk, mchunk, m]
            b2c = consts.tile([128, 2], F32)           # bias per m-chunk
            w3 = consts.tile([128, 2, D], DT)         # [k, kchunk, m]
            b3t = consts.tile([D2, 1], F32)            # 0.1*b3[:32]
            glsp = consts.tile([D2, 1], F32)           # gls[32:]
            swb = consts.tile([97, D], DT)   # rows 0:33 = SWS, 64:97 = SWO
            sws = swb[0:33, :]
            swo = swb[64:97, :]
            lj2 = consts.tile([D2, 1], DT)            # all 2.0
            ljc = consts.tile([1, 1], F32)             # sum(gls)

            nc.sync.dma_start(out=ident, in_=ident_d[:])
            nc.sync.dma_start(out=w1e, in_=w1e_d[:].rearrange("k (c m) -> k c m", c=2))
            nc.sync.dma_start(
                out=w2, in_=w2_d[:].rearrange("(c k) (g m) -> k c g m", c=2, g=2)
            )
            nc.sync.dma_start(out=b2c, in_=b2_d[:].rearrange("(c m) -> m c", c=2))
            nc.sync.dma_start(out=w3, in_=w3_d[:].rearrange("(c k) m -> k c m", c=2))
            nc.sync.dma_start(out=b3t, in_=b3t_d[:].rearrange("(k o) -> k o", o=1))
            nc.sync.dma_start(out=glsp, in_=glsp_d[:].rearrange("(k o) -> k o", o=1))
            nc.sync.dma_start(out=sws, in_=sws_d[:])
            nc.sync.dma_start(out=swo, in_=swo_d[:])
            nc.sync.dma_start(out=lj2, in_=lj2_d[:].rearrange("(k o) -> k o", o=1))
            nc.sync.dma_start(out=ljc, in_=ljc_d[:].rearrange("(k o) -> k o", o=1))

            # Persistent cross-phase buffers.
            # xT rows: 0:64 = x transposed (feature-major), 64 = ones
            xT = persist.tile([65, BL], DT)
            # cop rows: 0:32 = t (tanh, exp'd in place in phase 2);
            # 32 = ones; 64:96 = po; 96 = ones
            cop = persist.tile([97, BL], DT)
            ones_ap = ones_d[:].rearrange("(o n) -> o n", o=1)
            nc.sync.dma_start(out=xT[64:65, :], in_=ones_ap)
            nc.sync.dma_start(out=cop[32:33, :], in_=ones_ap)
            nc.sync.dma_start(out=cop[96:97, :], in_=ones_ap)

            # ---------------- Phase 1: MLP + tanh + logjac ----------------
            with (
                tc.tile_pool(name="xin", bufs=3) as xin,
                tc.tile_pool(name="hsb", bufs=3) as hsb,
                tc.tile_pool(name="psx", bufs=2, space="PSUM") as psx,
                tc.tile_pool(name="psh", bufs=2, space="PSUM") as psh,
                tc.tile_pool(name="psp", bufs=1, space="PSUM") as psp,
                tc.tile_pool(name="psl", bufs=1, space="PSUM") as psl,
            ):
                for i in range(TILES):
                    cols = slice(i * NT, (i + 1) * NT)
                    x_bt = xin.tile([NP, NCH, D], DT)
                    nc.sync.dma_start(out=x_bt, in_=x_ap[:, i * NCH:(i + 1) * NCH, :])

                    xps = psx.tile([D, NT], DT)
                    for n in range(NCH):
                        nc.tensor.transpose(
                            _tr(xps[:, n * NP:(n + 1) * NP]),
                            _tr(x_bt[:, n, :]), _tr(ident)
                        )
                    nc.vector.tensor_copy(xT[0:D, cols], xps)

                    rhs_x = _mm(xT[0:65, cols])
                    hps = psh.tile([128, 2 * NT], F32, tag="hps")
                    nc.tensor.matmul(hps[:, 0:NT], _mm(w1e[:, 0, :]), rhs_x,
                                     start=True, stop=True)
                    nc.tensor.matmul(hps[:, NT:2 * NT], _mm(w1e[:, 1, :]), rhs_x,
                                     start=True, stop=True)
                    h1 = hsb.tile([128, 2 * NT], DT, tag="h")
                    nc.scalar.activation(out=h1, in_=hps, func=gelu_f)

                    hps2 = psh.tile([128, 2 * NT], F32, tag="hps")
                    for m in range(2):
                        nc.tensor.matmul(hps2[:, m * NT:(m + 1) * NT],
                                         _mm(w2[:, 0, m, :]), _mm(h1[:, 0:NT]),
                                         start=True, stop=False)
                        nc.tensor.matmul(hps2[:, m * NT:(m + 1) * NT],
                                         _mm(w2[:, 1, m, :]), _mm(h1[:, NT:2 * NT]),
                                         start=False, stop=True)
                    h2 = hsb.tile([128, 2 * NT], DT, tag="h")
                    for m in range(2):
                        nc.scalar.activation(out=h2[:, m * NT:(m + 1) * NT],
                                             in_=hps2[:, m * NT:(m + 1) * NT],
                                             func=gelu_f, bias=b2c[:, m:m + 1])

                    pps = psp.tile([64, NT], F32)
                    nc.tensor.matmul(pps[0:D, :], _mm(w3[:, 0, :]), _mm(h2[:, 0:NT]),
                                     start=True, stop=False)
                    nc.tensor.matmul(pps[0:D, :], _mm(w3[:, 1, :]), _mm(h2[:, NT:2 * NT]),
                                     start=False, stop=True)

                    # t = tanh(0.1*params_t + 0.1*b3t)
                    nc.scalar.activation(out=cop[0:D2, cols], in_=pps[0:D2, :],
                                         func=tanh_f, bias=b3t, scale=0.1)
                    # po (raw)
                    nc.vector.tensor_copy(cop[64:64 + D2, cols], pps[D2:D, :])
                    # logjac partial: sum_t 2*t  (M=1 matmul)
                    ljps = psl.tile([1, NT], F32)
                    nc.tensor.matmul(ljps, _mm(lj2), _mm(cop[0:D2, cols]),
                                     start=True, stop=True)
                    # + sum(gls), into the logjac output staging
                    ljs = hsb.tile([1, NT], F32, tag="lj")
                    nc.vector.tensor_scalar_add(ljs, ljps, ljc[0:1, 0:1])
                    nc.sync.dma_start(out=lj_d[:, cols], in_=ljs)

            # ---------------- Phase 2: exp + output ----------------
            with (
                tc.tile_pool(name="ysb", bufs=3) as ysb,
                tc.tile_pool(name="ybt", bufs=3) as ybt,
                tc.tile_pool(name="pso", bufs=2, space="PSUM") as pso,
                tc.tile_pool(name="psy", bufs=2, space="PSUM") as psy,
            ):
                for i in range(TILES):
                    cols = slice(i * NT, (i + 1) * NT)
                    # S_c = exp(2*t + glsp), in place over t
                    nc.scalar.activation(out=cop[0:D2, cols], in_=cop[0:D2, cols],
                                         func=exp_f, bias=glsp, scale=2.0)
                    ops_s = pso.tile([D, NT], F32, tag="os")
                    ops_o = pso.tile([D, NT], F32, tag="oo")
                    nc.tensor.matmul(ops_s, _mm(sws), _mm(cop[0:33, cols]),
                                     start=True, stop=True)
                    nc.tensor.matmul(ops_o, _mm(swo), _mm(cop[64:97, cols]),
                                     start=True, stop=True, tile_position=(64, 0))

                    ytmp = ysb.tile([D, NT], F32, tag="yt")
                    nc.vector.tensor_mul(ytmp, xT[0:D, cols], ops_s)
                    yfm = ysb.tile([D, NT], DT, tag="yf")
                    nc.vector.tensor_add(yfm, ytmp, ops_o)

                    yps = psy.tile([128, NCH * D], DT)
                    for n in range(NCH):
                        nc.tensor.transpose(
                            _tr(yps[:, n * D:(n + 1) * D]),
                            _tr(yfm[:, n * NP:(n + 1) * NP]),
                            _tr(ident[0:D, 0:D]),
                        )
                    y_bt = ybt.tile([NP, NCH, D], F32)
                    nc.scalar.copy(out=y_bt, in_=yps)
                    nc.sync.dma_start(out=y_ap[:, i * NCH:(i + 1) * NCH, :], in_=y_bt)

    nc.compile()
    return nc


def make_consts(perm, W1, b1, W2, b2, W3, b3, g_log_scale, g_offset):
    ip = np.argsort(perm)
    gls = (4.0 * np.tanh(g_log_scale.astype(np.float64) / 4.0))
    W1e = np.zeros((65, H), np.float64)
    W1e[ip[:D1], :] = W1
    W1e[64, :] = b1
    SWS = np.zeros((33, D), np.float64)
    SWO = np.zeros((33, D), np.float64)
    for j in range(D):
        c = int(perm[j])
        a = np.exp(gls[c])
        if c >= D1:
            t = c - D1
            SWS[t, j] = 1.0
            SWO[t, j] = a
            SWO[32, j] = g_offset[c] + a * b3[D2 + t]
        else:
            SWS[32, j] = a
            SWO[32, j] = g_offset[c]
    f32 = np.float32
    return {
        "W1e": W1e.astype(f32),
        "W2": W2.astype(f32),
        "b2": b2.astype(f32),
        "W3r": W3.astype(f32),
        "b3t": (0.1 * b3[:D2]).astype(f32),
        "glsp": gls[D1:].astype(f32),
        "SWS": SWS.astype(f32),
        "SWO": SWO.astype(f32),
        "ident": np.eye(128, dtype=f32),
        "ones": np.ones((BL,), f32),
        "lj2": np.full((D2,), 2.0, f32),
        "ljc": np.array([gls.sum()], f32),
    }


_PROGRAM = None


def _ensure_ntff_hook():
    """Register the axon NTFF profile hook if boot didn't (shim path)."""
    import contextlib
    import ctypes
    try:
        from antenv import axon_hooks
    except ImportError:
        return
    if axon_hooks.get_axon_ntff_profile_hook() is not None:
        return
    so_path = "/opt/axon/libaxon_pjrt.so"
    if not os.path.exists(so_path):
        return
    lib = ctypes.CDLL(so_path)
    if not hasattr(lib, "axon_start_nrt_profile"):
        return
    lib.axon_start_nrt_profile.argtypes = [
        ctypes.POINTER(ctypes.c_int64), ctypes.c_size_t]
    lib.axon_start_nrt_profile.restype = ctypes.c_int64
    lib.axon_stop_nrt_profile.argtypes = [ctypes.c_char_p]
    lib.axon_stop_nrt_profile.restype = ctypes.c_int64

    @contextlib.contextmanager
    def _hook(output_dir, device_ids):
        import jax
        jax.devices()
        if device_ids:
            ids = (ctypes.c_int64 * len(device_ids))(*device_ids)
            rc = lib.axon_start_nrt_profile(ids, len(device_ids))
        else:
            rc = lib.axon_start_nrt_profile(None, 0)
        if rc != 0:
            raise RuntimeError(f"axon_start_nrt_profile rc={rc}")
        try:
            yield
        finally:
            n = lib.axon_stop_nrt_profile(str(output_dir).encode())
            print(f"profile: {n} file(s) written to {output_dir}")

    axon_hooks.set_axon_ntff_profile_hook(_hook)


def kernel(x, perm, W1, b1, W2, b2, W3, b3, g_log_scale, g_offset,
           _want_profile=False):
    global _PROGRAM
    x = np.ascontiguousarray(np.asarray(x, np.float32))
    consts = make_consts(np.asarray(perm), np.asarray(W1), np.asarray(b1),
                         np.asarray(W2), np.asarray(b2), np.asarray(W3),
                         np.asarray(b3), np.asarray(g_log_scale),
                         np.asarray(g_offset))
    if _PROGRAM is None:
        _PROGRAM = build_program()
    nc = _PROGRAM

    in_maps = []
    for c in range(NCORES):
        m = dict(consts)
        m["x"] = x[c * BL:(c + 1) * BL]
        in_maps.append(m)

    kw = {}
    if _want_profile:
        _ensure_ntff_hook()
        kw = dict(trace=True)
    res = run_bass_kernel_spmd(nc, in_maps, list(range(NCORES)), **kw)
    y = np.concatenate([res.results[c]["y"] for c in range(NCORES)], axis=0)
    lj = np.concatenate([res.results[c]["lj"].reshape(-1) for c in range(NCORES)])
    if _want_profile:
        return (y, lj), res
    return y, lj


if __name__ == "__main__":
    rng = np.random.default_rng(0)
    xs = rng.standard_normal((B, D)).astype(np.float32)
    pm = rng.permutation(D).astype(np.int32)
    out = kernel(
        xs, pm,
        rng.standard_normal((D1, H)).astype(np.float32) / np.sqrt(D1),
        np.zeros(H, np.float32),
        rng.standard_normal((H, H)).astype(np.float32) / np.sqrt(H),
        np.zeros(H, np.float32),
        rng.standard_normal((H, 2 * D2)).astype(np.float32) / np.sqrt(H),
        np.zeros(2 * D2, np.float32),
        (0.1 * rng.standard_normal(D)).astype(np.float32),
        (0.1 * rng.standard_normal(D)).astype(np.float32),
    )
    print("smoke ok", out[0].shape, out[1].shape)


# revision 16
# speedup vs baseline: 1.0560x; 1.0560x over previous
"""
Trainium2 Bass kernel for nn_AllInOneFlow (affine coupling flow).

Math folding (host side, exact):
  - inv_perm[perm[j]] == j  =>  the input gather and output scatter cancel:
    output column j reads input column j.  All permutation effects fold into
    small constant matrices applied via the PE:
      W1e  [65,256] : rows 0:64 = W1 rows scattered to inv_perm[:32] (rest 0),
                      row 64 = b1 (bias via ones-row trick)
      SWS  [33,64]  : Sg  = SWS.T @ [S_c; ones],  S_c = exp(2*tanh_t + gls[32+t])
      SWO  [33,64]  : OFFg= SWO.T @ [po; ones]
    y[:,j] = Sg[j]*x[:,j] + OFFg[j];  log_jac = sum_t 2*tanh_t + sum(gls)

Device layout: activations feature-major [feat, batch]; batch streamed in
tiles of 512 columns; 8 cores pure data-parallel over the batch.
Phase 1 (gelu/tanh ACT table set) computes the MLP + tanh + logjac for all
tiles; Phase 2 (exp table set) does exp, output matmuls, final elementwise,
and transposes back to batch-major.
"""

import os
import ml_dtypes
import numpy as np

import concourse.bass as bass
import concourse.bacc as bacc_mod
import concourse.tile as tile
from concourse import mybir
from concourse.bass_utils import run_bass_kernel_spmd

# Problem constants (hardcoded per harness contract)
B, D, H = 131072, 64, 256
D1 = D // 2
D2 = D - D1
ALPHA = 2.0
NCORES = 8
BL = B // NCORES          # rows per core = 16384
NT = 512                  # batch columns per tile
TILES = BL // NT          # 32
NP = 128                  # n-dim of the (n p) partition split
NCH = NT // NP            # 4 transpose chunks per tile

F32 = mybir.dt.float32
F32R = mybir.dt.float32r
BF16 = mybir.dt.bfloat16

# Tunables: DT is the dtype of everything on the matmul path
# (F32 = exact 4cyc/row, F32R = 1cyc/row at N>=256)
DT = F32R


def _mm(ap):
    return ap


def _tr(ap):
    return ap


def build_program():
    nc = bacc_mod.Bacc("TRN2", target_bir_lowering=False, debug=False)

    x_d = nc.declare_dram_parameter("x", [BL, D], DT, isOutput=False)
    w1e_d = nc.declare_dram_parameter("W1e", [65, H], DT, isOutput=False)
    w2_d = nc.declare_dram_parameter("W2", [H, H], BF16, isOutput=False)
    b2_d = nc.declare_dram_parameter("b2", [H], F32, isOutput=False)
    w3_d = nc.declare_dram_parameter("W3r", [H, D], BF16, isOutput=False)
    b3t_d = nc.declare_dram_parameter("b3t", [D2], F32, isOutput=False)
    glsp_d = nc.declare_dram_parameter("glsp", [D2], F32, isOutput=False)
    sws_d = nc.declare_dram_parameter("SWS", [33, D], DT, isOutput=False)
    swo_d = nc.declare_dram_parameter("SWO", [33, D], DT, isOutput=False)
    ident_d = nc.declare_dram_parameter("ident", [128, 128], DT, isOutput=False)
    ones_d = nc.declare_dram_parameter("ones", [BL], DT, isOutput=False)
    lj2_d = nc.declare_dram_parameter("lj2", [D2], DT, isOutput=False)
    ljc_d = nc.declare_dram_parameter("ljc", [1], F32, isOutput=False)

    y_d = nc.declare_dram_parameter("y", [BL, D], F32, isOutput=True)
    lj_d = nc.declare_dram_parameter("lj", [1, BL], F32, isOutput=True)

    # batch index = n*128 + p  ->  partition p, chunk n
    x_ap = x_d[:].rearrange("(n p) d -> p n d", p=NP)
    y_ap = y_d[:].rearrange("(n p) d -> p n d", p=NP)

    gelu_f = mybir.ActivationFunctionType.Gelu_apprx_tanh
    tanh_f = mybir.ActivationFunctionType.Tanh
    exp_f = mybir.ActivationFunctionType.Exp

    with tile.TileContext(nc) as tc:
        with (
            tc.tile_pool(name="consts", bufs=1) as consts,
            tc.tile_pool(name="persist", bufs=1) as persist,
        ):
            ident = consts.tile([128, 128], DT)
            w1e = consts.tile([65, 2, 128], DT)       # [k, mchunk, m]
            w2 = consts.tile([128, 2, 2, 128], BF16)    # [k, kchunk, mchunk, m]
            b2c = consts.tile([128, 2], F32)           # bias per m-chunk
            w3 = consts.tile([128, 2, D], BF16)         # [k, kchunk, m]
            b3t = consts.tile([D2, 1], F32)            # 0.1*b3[:32]
            glsp = consts.tile([D2, 1], F32)           # gls[32:]
            swb = consts.tile([97, D], DT)   # rows 0:33 = SWS, 64:97 = SWO
            sws = swb[0:33, :]
            swo = swb[64:97, :]
            lj2 = consts.tile([D2, 1], DT)            # all 2.0
            ljc = consts.tile([1, 1], F32)             # sum(gls)

            nc.sync.dma_start(out=ident, in_=ident_d[:])
            nc.sync.dma_start(out=w1e, in_=w1e_d[:].rearrange("k (c m) -> k c m", c=2))
            nc.sync.dma_start(
                out=w2, in_=w2_d[:].rearrange("(c k) (g m) -> k c g m", c=2, g=2)
            )
            nc.sync.dma_start(out=b2c, in_=b2_d[:].rearrange("(c m) -> m c", c=2))
            nc.sync.dma_start(out=w3, in_=w3_d[:].rearrange("(c k) m -> k c m", c=2))
            nc.sync.dma_start(out=b3t, in_=b3t_d[:].rearrange("(k o) -> k o", o=1))
            nc.sync.dma_start(out=glsp, in_=glsp_d[:].rearrange("(k o) -> k o", o=1))
            nc.sync.dma_start(out=sws, in_=sws_d[:])
            nc.sync.dma_start(out=swo, in_=swo_d[:])
            nc.sync.dma_start(out=lj2, in_=lj2_d[:].rearrange("(k o) -> k o", o=1))
            nc.sync.dma_start(out=ljc, in_=ljc_d[:].rearrange("(k o) -> k o", o=1))

            # Persistent cross-phase buffers.
            # xT rows: 0:64 = x transposed (feature-major), 64 = ones
            xT = persist.tile([65, BL], DT)
            # cop rows: 0:32 = t (tanh, exp'd in place in phase 2);
            # 32 = ones; 64:96 = po; 96 = ones
            cop = persist.tile([97, BL], DT)
            ones_ap = ones_d[:].rearrange("(o n) -> o n", o=1)
            nc.sync.dma_start(out=xT[64:65, :], in_=ones_ap)
            nc.sync.dma_start(out=cop[32:33, :], in_=ones_ap)
            nc.sync.dma_start(out=cop[96:97, :], in_=ones_ap)

            # ---------------- Phase 1: MLP + tanh + logjac ----------------
            with (
                tc.tile_pool(name="xin", bufs=3) as xin,
                tc.tile_pool(name="hsb", bufs=3) as hsb,
                tc.tile_pool(name="psx", bufs=2, space="PSUM") as psx,
                tc.tile_pool(name="psh", bufs=2, space="PSUM") as psh,
                tc.tile_pool(name="psp", bufs=2, space="PSUM") as psp,
            ):
                for i in range(TILES):
                    cols = slice(i * NT, (i + 1) * NT)
                    x_bt = xin.tile([NP, NCH, D], DT)
                    nc.sync.dma_start(out=x_bt, in_=x_ap[:, i * NCH:(i + 1) * NCH, :])

                    xps = psx.tile([D, NT], DT)
                    for n in range(NCH):
                        nc.tensor.transpose(
                            _tr(xps[:, n * NP:(n + 1) * NP]),
                            _tr(x_bt[:, n, :]), _tr(ident)
                        )
                    nc.vector.tensor_copy(xT[0:D, cols], xps)

                    rhs_x = _mm(xT[0:65, cols])
                    hps = psh.tile([128, 2 * NT], F32, tag="hps")
                    nc.tensor.matmul(hps[:, 0:NT], _mm(w1e[:, 0, :]), rhs_x,
                                     start=True, stop=True)
                    nc.tensor.matmul(hps[:, NT:2 * NT], _mm(w1e[:, 1, :]), rhs_x,
                                     start=True, stop=True)
                    h1 = hsb.tile([128, 2 * NT], BF16, tag="h")
                    nc.scalar.activation(out=h1, in_=hps, func=gelu_f)

                    hps2 = psh.tile([128, 2 * NT], F32, tag="hps")
                    for m in range(2):
                        nc.tensor.matmul(hps2[:, m * NT:(m + 1) * NT],
                                         _mm(w2[:, 0, m, :]), _mm(h1[:, 0:NT]),
                                         start=True, stop=False)
                        nc.tensor.matmul(hps2[:, m * NT:(m + 1) * NT],
                                         _mm(w2[:, 1, m, :]), _mm(h1[:, NT:2 * NT]),
                                         start=False, stop=True)
                    h2 = hsb.tile([128, 2 * NT], BF16, tag="h")
                    for m in range(2):
                        nc.scalar.activation(out=h2[:, m * NT:(m + 1) * NT],
                                             in_=hps2[:, m * NT:(m + 1) * NT],
                                             func=gelu_f, bias=b2c[:, m:m + 1])

                    pps = psp.tile([64, NT], F32)
                    nc.tensor.matmul(pps[0:D, :], _mm(w3[:, 0, :]), _mm(h2[:, 0:NT]),
                                     start=True, stop=False)
                    nc.tensor.matmul(pps[0:D, :], _mm(w3[:, 1, :]), _mm(h2[:, NT:2 * NT]),
                                     start=False, stop=True)

                    # raw params_t and po -> cop (tanh happens in phase 2,
                    # so phase 1 stays entirely on the gelu table set)
                    nc.vector.tensor_copy(cop[0:D2, cols], pps[0:D2, :])
                    nc.vector.tensor_copy(cop[64:64 + D2, cols], pps[D2:D, :])

            # ---------------- Phase 2: exp + output ----------------
            with (
                tc.tile_pool(name="ysb", bufs=3) as ysb,
                tc.tile_pool(name="ybt", bufs=3) as ybt,
                tc.tile_pool(name="pso", bufs=2, space="PSUM") as pso,
                tc.tile_pool(name="psy", bufs=2, space="PSUM") as psy,
                tc.tile_pool(name="psl", bufs=1, space="PSUM") as psl,
                tc.tile_pool(name="psq", bufs=1, space="PSUM") as psq,
            ):
                w2flat = w2.rearrange("k c g m -> k (c g m)")
                for i in range(TILES):
                    cols = slice(i * NT, (i + 1) * NT)
                    # bf16 HAM heater: keeps the PE clock gate at 8/8 during
                    # the fp32r-heavy output phase (fp32-mode matmuls do not
                    # register as PE activity for the HAM monitor)
                    heat = psq.tile([128, NT], F32, tag="heat")
                    nc.tensor.matmul(heat, w2[:, 0, 0, :], w2flat,
                                     start=True, stop=True)
                    # t = tanh(0.1*params_t + 0.1*b3t), in place
                    nc.scalar.activation(out=cop[0:D2, cols], in_=cop[0:D2, cols],
                                         func=tanh_f, bias=b3t, scale=0.1)
                    # logjac = sum_t 2*t + sum(gls)
                    ljps = psl.tile([1, NT], F32)
                    nc.tensor.matmul(ljps, _mm(lj2), _mm(cop[0:D2, cols]),
                                     start=True, stop=True)
                    ljs = ysb.tile([1, NT], F32, tag="lj")
                    nc.vector.tensor_scalar_add(ljs, ljps, ljc[0:1, 0:1])
                    nc.sync.dma_start(out=lj_d[:, cols], in_=ljs)
                    # S_c = exp(2*t + glsp), in place over t
                    nc.scalar.activation(out=cop[0:D2, cols], in_=cop[0:D2, cols],
                                         func=exp_f, bias=glsp, scale=2.0)
                    ops_s = pso.tile([D, NT], F32, tag="os")
                    ops_o = pso.tile([D, NT], F32, tag="oo")
                    nc.tensor.matmul(ops_s, _mm(sws), _mm(cop[0:33, cols]),
                                     start=True, stop=True)
                    nc.tensor.matmul(ops_o, _mm(swo), _mm(cop[64:97, cols]),
                                     start=True, stop=True, tile_position=(64, 0))

                    ytmp = ysb.tile([D, NT], F32, tag="yt")
                    nc.vector.tensor_mul(ytmp, xT[0:D, cols], ops_s)
                    yfm = ysb.tile([D, NT], DT, tag="yf")
                    nc.vector.tensor_add(yfm, ytmp, ops_o)

                    yps = psy.tile([128, NCH * D], DT)
                    for n in range(NCH):
                        nc.tensor.transpose(
                            _tr(yps[:, n * D:(n + 1) * D]),
                            _tr(yfm[:, n * NP:(n + 1) * NP]),
                            _tr(ident[0:D, 0:D]),
                        )
                    y_bt = ybt.tile([NP, NCH, D], F32)
                    nc.scalar.copy(out=y_bt, in_=yps)
                    nc.sync.dma_start(out=y_ap[:, i * NCH:(i + 1) * NCH, :], in_=y_bt)

    nc.compile()
    return nc


def make_consts(perm, W1, b1, W2, b2, W3, b3, g_log_scale, g_offset):
    ip = np.argsort(perm)
    gls = (4.0 * np.tanh(g_log_scale.astype(np.float64) / 4.0))
    W1e = np.zeros((65, H), np.float64)
    W1e[ip[:D1], :] = W1
    W1e[64, :] = b1
    SWS = np.zeros((33, D), np.float64)
    SWO = np.zeros((33, D), np.float64)
    for j in range(D):
        c = int(perm[j])
        a = np.exp(gls[c])
        if c >= D1:
            t = c - D1
            SWS[t, j] = 1.0
            SWO[t, j] = a
            SWO[32, j] = g_offset[c] + a * b3[D2 + t]
        else:
            SWS[32, j] = a
            SWO[32, j] = g_offset[c]
    f32 = np.float32
    return {
        "W1e": W1e.astype(f32),
        "W2": W2.astype(ml_dtypes.bfloat16),
        "b2": b2.astype(f32),
        "W3r": W3.astype(ml_dtypes.bfloat16),
        "b3t": (0.1 * b3[:D2]).astype(f32),
        "glsp": gls[D1:].astype(f32),
        "SWS": SWS.astype(f32),
        "SWO": SWO.astype(f32),
        "ident": np.eye(128, dtype=f32),
        "ones": np.ones((BL,), f32),
        "lj2": np.full((D2,), 2.0, f32),
        "ljc": np.array([gls.sum()], f32),
    }


_PROGRAM = None


def _ensure_ntff_hook():
    """Register the axon NTFF profile hook if boot didn't (shim path)."""
    import contextlib
    import ctypes
    try:
        from antenv import axon_hooks
    except ImportError:
        return
    if axon_hooks.get_axon_ntff_profile_hook() is not None:
        return
    so_path = "/opt/axon/libaxon_pjrt.so"
    if not os.path.exists(so_path):
        return
    lib = ctypes.CDLL(so_path)
    if not hasattr(lib, "axon_start_nrt_profile"):
        return
    lib.axon_start_nrt_profile.argtypes = [
        ctypes.POINTER(ctypes.c_int64), ctypes.c_size_t]
    lib.axon_start_nrt_profile.restype = ctypes.c_int64
    lib.axon_stop_nrt_profile.argtypes = [ctypes.c_char_p]
    lib.axon_stop_nrt_profile.restype = ctypes.c_int64

    @contextlib.contextmanager
    def _hook(output_dir, device_ids):
        import jax
        jax.devices()
        if device_ids:
            ids = (ctypes.c_int64 * len(device_ids))(*device_ids)
            rc = lib.axon_start_nrt_profile(ids, len(device_ids))
        else:
            rc = lib.axon_start_nrt_profile(None, 0)
        if rc != 0:
            raise RuntimeError(f"axon_start_nrt_profile rc={rc}")
        try:
            yield
        finally:
            n = lib.axon_stop_nrt_profile(str(output_dir).encode())
            print(f"profile: {n} file(s) written to {output_dir}")

    axon_hooks.set_axon_ntff_profile_hook(_hook)


def kernel(x, perm, W1, b1, W2, b2, W3, b3, g_log_scale, g_offset,
           _want_profile=False):
    global _PROGRAM
    x = np.ascontiguousarray(np.asarray(x, np.float32))
    consts = make_consts(np.asarray(perm), np.asarray(W1), np.asarray(b1),
                         np.asarray(W2), np.asarray(b2), np.asarray(W3),
                         np.asarray(b3), np.asarray(g_log_scale),
                         np.asarray(g_offset))
    if _PROGRAM is None:
        _PROGRAM = build_program()
    nc = _PROGRAM

    in_maps = []
    for c in range(NCORES):
        m = dict(consts)
        m["x"] = x[c * BL:(c + 1) * BL]
        in_maps.append(m)

    kw = {}
    if _want_profile:
        _ensure_ntff_hook()
        kw = dict(trace=True)
    res = run_bass_kernel_spmd(nc, in_maps, list(range(NCORES)), **kw)
    y = np.concatenate([res.results[c]["y"] for c in range(NCORES)], axis=0)
    lj = np.concatenate([res.results[c]["lj"].reshape(-1) for c in range(NCORES)])
    if _want_profile:
        return (y, lj), res
    return y, lj


if __name__ == "__main__":
    rng = np.random.default_rng(0)
    xs = rng.standard_normal((B, D)).astype(np.float32)
    pm = rng.permutation(D).astype(np.int32)
    out = kernel(
        xs, pm,
        rng.standard_normal((D1, H)).astype(np.float32) / np.sqrt(D1),
        np.zeros(H, np.float32),
        rng.standard_normal((H, H)).astype(np.float32) / np.sqrt(H),
        np.zeros(H, np.float32),
        rng.standard_normal((H, 2 * D2)).astype(np.float32) / np.sqrt(H),
        np.zeros(2 * D2, np.float32),
        (0.1 * rng.standard_normal(D)).astype(np.float32),
        (0.1 * rng.standard_normal(D)).astype(np.float32),
    )
    print("smoke ok", out[0].shape, out[1].shape)


# revision 18
# speedup vs baseline: 1.1398x; 1.0794x over previous
"""
Trainium2 Bass kernel for nn_AllInOneFlow (affine coupling flow).

Math folding (host side, exact):
  - inv_perm[perm[j]] == j  =>  the input gather and output scatter cancel:
    output column j reads input column j.  All permutation effects fold into
    small constant matrices applied via the PE:
      W1e  [65,256] : rows 0:64 = W1 rows scattered to inv_perm[:32] (rest 0),
                      row 64 = b1 (bias via ones-row trick)
      SWS  [33,64]  : Sg  = SWS.T @ [S_c; ones],  S_c = exp(2*tanh_t + gls[32+t])
      SWO  [33,64]  : OFFg= SWO.T @ [po; ones]
    y[:,j] = Sg[j]*x[:,j] + OFFg[j];  log_jac = sum_t 2*tanh_t + sum(gls)

Device layout: activations feature-major [feat, batch]; batch streamed in
tiles of 512 columns; 8 cores pure data-parallel over the batch.
Phase 1 (gelu/tanh ACT table set) computes the MLP + tanh + logjac for all
tiles; Phase 2 (exp table set) does exp, output matmuls, final elementwise,
and transposes back to batch-major.
"""

import os
import ml_dtypes
import numpy as np

import concourse.bass as bass
import concourse.bacc as bacc_mod
import concourse.tile as tile
from concourse import mybir
from concourse.bass_utils import run_bass_kernel_spmd

# Problem constants (hardcoded per harness contract)
B, D, H = 131072, 64, 256
D1 = D // 2
D2 = D - D1
ALPHA = 2.0
NCORES = 8
BL = B // NCORES          # rows per core = 16384
NT = 512                  # batch columns per tile
TILES = BL // NT          # 32
NP = 128                  # n-dim of the (n p) partition split
NCH = NT // NP            # 4 transpose chunks per tile

F32 = mybir.dt.float32
F32R = mybir.dt.float32r
BF16 = mybir.dt.bfloat16

# Tunables: DT is the dtype of everything on the matmul path
# (F32 = exact 4cyc/row, F32R = 1cyc/row at N>=256)
DT = F32R


def _mm(ap):
    return ap


def _tr(ap):
    return ap


def build_program():
    nc = bacc_mod.Bacc("TRN2", target_bir_lowering=False, debug=False)

    x_d = nc.declare_dram_parameter("x", [BL, D], DT, isOutput=False)
    w1e_d = nc.declare_dram_parameter("W1e", [65, H], DT, isOutput=False)
    w2_d = nc.declare_dram_parameter("W2", [H, H], BF16, isOutput=False)
    b2_d = nc.declare_dram_parameter("b2", [H], F32, isOutput=False)
    w3_d = nc.declare_dram_parameter("W3r", [H, D], BF16, isOutput=False)
    b3t2_d = nc.declare_dram_parameter("b3t2", [D], F32, isOutput=False)
    glsp2_d = nc.declare_dram_parameter("glsp2", [D], F32, isOutput=False)
    swsd_d = nc.declare_dram_parameter("SWSd", [D, D], BF16, isOutput=False)
    swod_d = nc.declare_dram_parameter("SWOd", [D, D], BF16, isOutput=False)
    ccol_d = nc.declare_dram_parameter("Ccol", [D], F32, isOutput=False)
    bcol_d = nc.declare_dram_parameter("Bcol", [D], F32, isOutput=False)
    ident_d = nc.declare_dram_parameter("ident", [128, 128], DT, isOutput=False)
    ones_d = nc.declare_dram_parameter("ones", [BL], DT, isOutput=False)
    lj22_d = nc.declare_dram_parameter("lj22", [D, 2], DT, isOutput=False)
    ljc_d = nc.declare_dram_parameter("ljc", [2], F32, isOutput=False)

    y_d = nc.declare_dram_parameter("y", [BL, D], F32, isOutput=True)
    lj_d = nc.declare_dram_parameter("lj", [1, BL], F32, isOutput=True)

    # batch index = n*128 + p  ->  partition p, chunk n
    x_ap = x_d[:].rearrange("(n p) d -> p n d", p=NP)
    y_ap = y_d[:].rearrange("(n p) d -> p n d", p=NP)

    gelu_f = mybir.ActivationFunctionType.Gelu_apprx_tanh
    tanh_f = mybir.ActivationFunctionType.Tanh
    exp_f = mybir.ActivationFunctionType.Exp
    ADD = mybir.AluOpType.add
    MULT = mybir.AluOpType.mult

    with tile.TileContext(nc) as tc:
        with (
            tc.tile_pool(name="consts", bufs=1) as consts,
            tc.tile_pool(name="persist", bufs=1) as persist,
        ):
            ident = consts.tile([128, 128], DT)
            w1e = consts.tile([65, 2, 128], DT)        # [k, mchunk, m]
            w2 = consts.tile([128, 2, 2, 128], BF16)   # [k, kchunk, mchunk, m]
            b2c = consts.tile([128, 2], F32)           # gelu2 bias per m-chunk
            w3 = consts.tile([128, 2, D], BF16)        # [k, kchunk, m]
            b3t2 = consts.tile([D, 1], F32)            # 0.1*b3t, duplicated rows
            glsp2 = consts.tile([D, 1], F32)           # gls[32:], duplicated rows
            swsd = consts.tile([D, D], BF16)           # SWS32 stacked twice
            swod = consts.tile([D, D], BF16)           # SWO32 stacked twice
            ccol = consts.tile([D, 1], F32)            # identity-col scale
            bcol = consts.tile([D, 1], F32)            # additive consts
            lj22 = consts.tile([D, 2], DT)             # logjac pair weights
            ljc = consts.tile([2, 1], F32)             # sum(gls), x2 rows

            nc.sync.dma_start(out=ident, in_=ident_d[:])
            nc.sync.dma_start(out=w1e, in_=w1e_d[:].rearrange("k (c m) -> k c m", c=2))
            nc.sync.dma_start(
                out=w2, in_=w2_d[:].rearrange("(c k) (g m) -> k c g m", c=2, g=2)
            )
            nc.sync.dma_start(out=b2c, in_=b2_d[:].rearrange("(c m) -> m c", c=2))
            nc.sync.dma_start(out=w3, in_=w3_d[:].rearrange("(c k) m -> k c m", c=2))
            nc.sync.dma_start(out=b3t2, in_=b3t2_d[:].rearrange("(k o) -> k o", o=1))
            nc.sync.dma_start(out=glsp2, in_=glsp2_d[:].rearrange("(k o) -> k o", o=1))
            nc.sync.dma_start(out=swsd, in_=swsd_d[:])
            nc.sync.dma_start(out=swod, in_=swod_d[:])
            nc.sync.dma_start(out=ccol, in_=ccol_d[:].rearrange("(k o) -> k o", o=1))
            nc.sync.dma_start(out=bcol, in_=bcol_d[:].rearrange("(k o) -> k o", o=1))
            nc.sync.dma_start(out=lj22, in_=lj22_d[:])
            nc.sync.dma_start(out=ljc, in_=ljc_d[:].rearrange("(k o) -> k o", o=1))

            # Persistent cross-phase buffers.
            # xT rows: 0:64 = x transposed (feature-major), 64 = ones
            xT = persist.tile([65, BL], DT)
            # Pair-packed params: pair k columns [512k, 512k+512);
            # rows 0:32 = tile 2k, rows 32:64 = tile 2k+1
            cop = persist.tile([D, BL // 2], DT)       # raw params_t -> tanh in ph2
            pob = persist.tile([D, BL // 2], BF16)     # raw po (bf16)
            ones_ap = ones_d[:].rearrange("(o n) -> o n", o=1)
            nc.sync.dma_start(out=xT[64:65, :], in_=ones_ap)

            # ---------------- Phase 1: transposes + MLP ----------------
            with (
                tc.tile_pool(name="xin", bufs=3) as xin,
                tc.tile_pool(name="hsb", bufs=3) as hsb,
                tc.tile_pool(name="psx", bufs=2, space="PSUM") as psx,
                tc.tile_pool(name="psh", bufs=2, space="PSUM") as psh,
                tc.tile_pool(name="psp", bufs=2, space="PSUM") as psp,
            ):
                for i in range(TILES):
                    cols = slice(i * NT, (i + 1) * NT)
                    par = i % 2
                    pcols = slice((i // 2) * NT, (i // 2 + 1) * NT)
                    prow = slice(par * D2, par * D2 + D2)
                    x_bt = xin.tile([NP, NCH, D], DT)
                    nc.sync.dma_start(out=x_bt, in_=x_ap[:, i * NCH:(i + 1) * NCH, :])

                    xps = psx.tile([D, NT], DT)
                    for n in range(NCH):
                        nc.tensor.transpose(
                            _tr(xps[:, n * NP:(n + 1) * NP]),
                            _tr(x_bt[:, n, :]), _tr(ident)
                        )
                    nc.vector.tensor_copy(xT[0:D, cols], xps)

                    rhs_x = _mm(xT[0:65, cols])
                    hps = psh.tile([128, 2 * NT], F32, tag="hps")
                    nc.tensor.matmul(hps[:, 0:NT], _mm(w1e[:, 0, :]), rhs_x,
                                     start=True, stop=True)
                    nc.tensor.matmul(hps[:, NT:2 * NT], _mm(w1e[:, 1, :]), rhs_x,
                                     start=True, stop=True)
                    h1 = hsb.tile([128, 2 * NT], BF16, tag="h")
                    nc.scalar.activation(out=h1, in_=hps, func=gelu_f)

                    hps2 = psh.tile([128, 2 * NT], F32, tag="hps")
                    for m in range(2):
                        nc.tensor.matmul(hps2[:, m * NT:(m + 1) * NT],
                                         w2[:, 0, m, :], h1[:, 0:NT],
                                         start=True, stop=False)
                        nc.tensor.matmul(hps2[:, m * NT:(m + 1) * NT],
                                         w2[:, 1, m, :], h1[:, NT:2 * NT],
                                         start=False, stop=True)
                    h2 = hsb.tile([128, 2 * NT], BF16, tag="h")
                    for m in range(2):
                        nc.scalar.activation(out=h2[:, m * NT:(m + 1) * NT],
                                             in_=hps2[:, m * NT:(m + 1) * NT],
                                             func=gelu_f, bias=b2c[:, m:m + 1])

                    pps = psp.tile([64, NT], F32)
                    nc.tensor.matmul(pps[0:D, :], w3[:, 0, :], h2[:, 0:NT],
                                     start=True, stop=False)
                    nc.tensor.matmul(pps[0:D, :], w3[:, 1, :], h2[:, NT:2 * NT],
                                     start=False, stop=True)

                    # raw params -> pair-packed staging (tanh/exp in phase 2)
                    nc.vector.tensor_copy(cop[prow, pcols], pps[0:D2, :])
                    nc.vector.tensor_copy(pob[prow, pcols], pps[D2:D, :])

            # ---------------- Phase 2: tanh/exp + output ----------------
            with (
                tc.tile_pool(name="ysb", bufs=4) as ysb,
                tc.tile_pool(name="sbf", bufs=2) as sbf,
                tc.tile_pool(name="ybt", bufs=3) as ybt,
                tc.tile_pool(name="pso", bufs=2, space="PSUM") as pso,
                tc.tile_pool(name="psy", bufs=2, space="PSUM") as psy,
                tc.tile_pool(name="psl", bufs=1, space="PSUM") as psl,
                tc.tile_pool(name="psq", bufs=1, space="PSUM") as psq,
            ):
                w2flat = w2.rearrange("k c g m -> k (c g m)")
                for k in range(TILES // 2):
                    pcols = slice(k * NT, (k + 1) * NT)
                    # bf16 HAM heater (fp32-mode matmuls don't register as
                    # PE activity, so the clock gate would drop to 4/8)
                    heat = psq.tile([128, NT], F32, tag="heat")
                    nc.tensor.matmul(heat, w2[:, 0, 0, :], w2flat,
                                     start=True, stop=True)
                    # t = tanh(0.1*p + 0.1*b3t) for the pair, in place
                    nc.scalar.activation(out=cop[:, pcols], in_=cop[:, pcols],
                                         func=tanh_f, bias=b3t2, scale=0.1)
                    # logjac for both tiles in one matmul
                    ljps = psl.tile([2, NT], F32)
                    nc.tensor.matmul(ljps, _mm(lj22), _mm(cop[:, pcols]),
                                     start=True, stop=True)
                    ljs = ysb.tile([2, NT], F32, tag="lj")
                    nc.vector.tensor_scalar_add(ljs, ljps, ljc)
                    nc.sync.dma_start(
                        out=lj_d[:, 2 * k * NT:(2 * k + 2) * NT]
                        .rearrange("o (r c) -> (o r) c", r=2),
                        in_=ljs)
                    # S = exp(2*t + glsp) for the pair -> bf16
                    s_bf = sbf.tile([D, NT], BF16, tag="S")
                    nc.scalar.activation(out=s_bf, in_=cop[:, pcols],
                                         func=exp_f, bias=glsp2, scale=2.0)

                    for par in range(2):
                        i = 2 * k + par
                        cols = slice(i * NT, (i + 1) * NT)
                        prow = slice(par * D2, par * D2 + D2)
                        ops_s = pso.tile([D, NT], F32, tag="os")
                        ops_o = pso.tile([D, NT], F32, tag="oo")
                        nc.tensor.matmul(ops_s, swsd[prow, :], s_bf[prow, :],
                                         start=True, stop=True,
                                         tile_position=(par * D2, 0))
                        nc.tensor.matmul(ops_o, swod[prow, :], pob[prow, pcols],
                                         start=True, stop=True,
                                         tile_position=(par * D2, 0))

                        # ytmp = (Sg + Ccol) * x ;  y = (OFFg + Bcol) + ytmp
                        ytmp = ysb.tile([D, NT], F32, tag="yt")
                        nc.vector.scalar_tensor_tensor(
                            ytmp, ops_s, ccol, xT[0:D, cols], op0=ADD, op1=MULT)
                        yfm = ysb.tile([D, NT], DT, tag="yf")
                        nc.vector.scalar_tensor_tensor(
                            yfm, ops_o, bcol, ytmp, op0=ADD, op1=ADD)

                        yps = psy.tile([128, NCH * D], DT)
                        for n in range(NCH):
                            nc.tensor.transpose(
                                _tr(yps[:, n * D:(n + 1) * D]),
                                _tr(yfm[:, n * NP:(n + 1) * NP]),
                                _tr(ident[0:D, 0:D]),
                            )
                        y_bt = ybt.tile([NP, NCH, D], F32)
                        nc.scalar.copy(out=y_bt, in_=yps)
                        nc.sync.dma_start(out=y_ap[:, i * NCH:(i + 1) * NCH, :],
                                          in_=y_bt)

    nc.compile()
    return nc


def make_consts(perm, W1, b1, W2, b2, W3, b3, g_log_scale, g_offset):
    ip = np.argsort(perm)
    gls = (4.0 * np.tanh(g_log_scale.astype(np.float64) / 4.0))
    W1e = np.zeros((65, H), np.float64)
    W1e[ip[:D1], :] = W1
    W1e[64, :] = b1
    SWS = np.zeros((D2, D), np.float64)
    SWO = np.zeros((D2, D), np.float64)
    Ccol = np.zeros((D,), np.float64)
    Bcol = np.zeros((D,), np.float64)
    for j in range(D):
        c = int(perm[j])
        a = np.exp(gls[c])
        if c >= D1:
            t = c - D1
            SWS[t, j] = 1.0
            SWO[t, j] = a
            Bcol[j] = g_offset[c] + a * b3[D2 + t]
        else:
            Ccol[j] = a
            Bcol[j] = g_offset[c]
    lj22 = np.zeros((D, 2), np.float64)
    lj22[:D2, 0] = 2.0
    lj22[D2:, 1] = 2.0
    f32 = np.float32
    bf16 = ml_dtypes.bfloat16
    return {
        "W1e": W1e.astype(f32),
        "W2": W2.astype(bf16),
        "b2": b2.astype(f32),
        "W3r": W3.astype(bf16),
        "b3t2": np.concatenate([0.1 * b3[:D2], 0.1 * b3[:D2]]).astype(f32),
        "glsp2": np.concatenate([gls[D1:], gls[D1:]]).astype(f32),
        "SWSd": np.vstack([SWS, SWS]).astype(bf16),
        "SWOd": np.vstack([SWO, SWO]).astype(bf16),
        "Ccol": Ccol.astype(f32),
        "Bcol": Bcol.astype(f32),
        "ident": np.eye(128, dtype=f32),
        "ones": np.ones((BL,), f32),
        "lj22": lj22.astype(f32),
        "ljc": np.array([gls.sum(), gls.sum()], f32),
    }


_PROGRAM = None


def _ensure_ntff_hook():
    """Register the axon NTFF profile hook if boot didn't (shim path)."""
    import contextlib
    import ctypes
    try:
        from antenv import axon_hooks
    except ImportError:
        return
    if axon_hooks.get_axon_ntff_profile_hook() is not None:
        return
    so_path = "/opt/axon/libaxon_pjrt.so"
    if not os.path.exists(so_path):
        return
    lib = ctypes.CDLL(so_path)
    if not hasattr(lib, "axon_start_nrt_profile"):
        return
    lib.axon_start_nrt_profile.argtypes = [
        ctypes.POINTER(ctypes.c_int64), ctypes.c_size_t]
    lib.axon_start_nrt_profile.restype = ctypes.c_int64
    lib.axon_stop_nrt_profile.argtypes = [ctypes.c_char_p]
    lib.axon_stop_nrt_profile.restype = ctypes.c_int64

    @contextlib.contextmanager
    def _hook(output_dir, device_ids):
        import jax
        jax.devices()
        if device_ids:
            ids = (ctypes.c_int64 * len(device_ids))(*device_ids)
            rc = lib.axon_start_nrt_profile(ids, len(device_ids))
        else:
            rc = lib.axon_start_nrt_profile(None, 0)
        if rc != 0:
            raise RuntimeError(f"axon_start_nrt_profile rc={rc}")
        try:
            yield
        finally:
            n = lib.axon_stop_nrt_profile(str(output_dir).encode())
            print(f"profile: {n} file(s) written to {output_dir}")

    axon_hooks.set_axon_ntff_profile_hook(_hook)


def kernel(x, perm, W1, b1, W2, b2, W3, b3, g_log_scale, g_offset,
           _want_profile=False):
    global _PROGRAM
    x = np.ascontiguousarray(np.asarray(x, np.float32))
    consts = make_consts(np.asarray(perm), np.asarray(W1), np.asarray(b1),
                         np.asarray(W2), np.asarray(b2), np.asarray(W3),
                         np.asarray(b3), np.asarray(g_log_scale),
                         np.asarray(g_offset))
    if _PROGRAM is None:
        _PROGRAM = build_program()
    nc = _PROGRAM

    in_maps = []
    for c in range(NCORES):
        m = dict(consts)
        m["x"] = x[c * BL:(c + 1) * BL]
        in_maps.append(m)

    kw = {}
    if _want_profile:
        _ensure_ntff_hook()
        kw = dict(trace=True)
    res = run_bass_kernel_spmd(nc, in_maps, list(range(NCORES)), **kw)
    y = np.concatenate([res.results[c]["y"] for c in range(NCORES)], axis=0)
    lj = np.concatenate([res.results[c]["lj"].reshape(-1) for c in range(NCORES)])
    if _want_profile:
        return (y, lj), res
    return y, lj


if __name__ == "__main__":
    rng = np.random.default_rng(0)
    xs = rng.standard_normal((B, D)).astype(np.float32)
    pm = rng.permutation(D).astype(np.int32)
    out = kernel(
        xs, pm,
        rng.standard_normal((D1, H)).astype(np.float32) / np.sqrt(D1),
        np.zeros(H, np.float32),
        rng.standard_normal((H, H)).astype(np.float32) / np.sqrt(H),
        np.zeros(H, np.float32),
        rng.standard_normal((H, 2 * D2)).astype(np.float32) / np.sqrt(H),
        np.zeros(2 * D2, np.float32),
        (0.1 * rng.standard_normal(D)).astype(np.float32),
        (0.1 * rng.standard_normal(D)).astype(np.float32),
    )
    print("smoke ok", out[0].shape, out[1].shape)
